# revision 60
# baseline (speedup 1.0000x reference)
"""Trainium2 Bass kernel for nn_Attention_35708358099413.

Reference computation (T=8192, B=64, H=256, N=128):
    sW     = s_before @ W.T + b                      # [1,B,H]
    denom  = einsum('obd,tbd->ob', sW, h)            # [1,B] (sum over T and H)
    scores = einsum('obd,nbd->obn', sW, h_sliced) / denom
    c_t    = (scores.T * h_sliced).sum(0)            # [B,H]

Strategy: pure data-parallel over batch. 8 cores x 8 batches each; no
collectives. Per core the dominant work is h_sum[b,d] = sum_t h[t,b,d].

Precision split (load-bearing): h's ONLY use is the per-batch scalar
denom (min |denom| = 210 on the real inputs), whose error scales each
batch's output UNIFORMLY - so h tolerates fp16 (host-side cast halves
the stream to 32MB/core; measured 2.1e-3 output relmax). The c_t chain
(h_sliced, scores, ps_o) must stay strictly fp32: min |expected
output| = 4e-5 with the harness's elementwise rel-err metric, so even
1e-5-level ABSOLUTE noise there fails.

Per-core pipeline (hprec='f16', pairs=1 default):
  - h [T, 8*256] fp16 viewed [8, 128, 8, 2048]; 4MB tiles (32KB per
    partition, the efficient descriptor size) stream on the two HWDGE
    rings (sync/scalar alternating). The last tile of EACH ring is
    chunked (512KB) so its consumers trail chunk-wise. Ring loads are
    balanced: w + half of hs ride sync, s/b + other hs half scalar
    (16.75 vs 16.51 MB).
  - Walrus emits an LDWEIGHTS per matmul (--enable-ldw-opt=false), so
    PE-only reduction costs 64x(124+107)ns = 15us/tile > 12us DMA.
    Instead the DVE adds chunk PAIRS in fp16 first (all-2-byte SBUF
    operands hit the DVE 4x mode, ~0.6us per [128,2048] add), then the
    PE consumes pairs: 32 matmuls/tile (~7.4us). Each h element takes
    exactly one extra fp16 rounding: denom relmax 3.3e-3 (simulated on
    the real inputs), 6x inside the 2e-2 gate.
  - PE matmul trick: lhsT = e3[:, b, :] (ones in column b) lands batch
    b's column sums on PSUM partition b, accumulating into one [8, 256]
    PSUM tile across all tiles. cp-outer ordering means the tail trails
    chunk-wise (LDW cost is per-matmul anyway, so ordering is free);
    the rings' final chunks split into batch-aligned quarters/eighths so
    only 2 matmuls follow the very last 64KB.
  - sW = s @ W.T + b on PE from on-chip transposes of s and W, emitted
    mid-stream. sW is broadcast to all 128 partitions by placing it
    block-diagonally ([8, 8*256], DVE mask multiply) and multiplying by
    ones8 on PE - no DRAM bounce.
  - scores_raw[n,b] = rowwise reduce of (h_sliced * bcast_sW) on DVE
    (tile 3); c_raw[b,:] = scores^T @ h_sliced on PE via masked score
    columns (tile 5, so its fp32 matmuls never sit in the tail).
    denom[b] = <sW[b], h_sum[b]> and the 1/denom scale fold in at the
    very end (~2us tail).

hprec='f32r' pairs=0 reproduces the previous all-fp32 kernel (~188us);
f16+pairs measures ~102us (fast draw) against a ~92.5us stream floor
(360 GB/s per-core DMA cap = 2 rings x 8 engines x 22.5 GB/s; ~7us
framework start + ~3.5us tail are the rest). Exec is bimodal: slow
draws ~118us trace to an 11us initial cross-engine barrier
(sibling-core launch skew), not to anything in this program.

Negative results (do not retry): deleting "redundant" Ldweights via a
BIR post-pass compiles but yields garbage on hardware - the PE does
not retain the stationary operand without each matmul's paired
Ldweights (and PE time wasn't the bottleneck anyway).
tensor_tensor_reduce crashes walrus codegen ("ISA wrong length").
"""

import json

import numpy as np

T, B, H, N = 8192, 64, 256, 128
NCORES = 8
BL = B // NCORES          # 8 batches per core
F = BL * H                # 2048

_CACHE = {}


def _split_multi_waits(bir_bytes, max_waits=1):
    """Walrus in some containers rejects instructions carrying more than
    one sem wait ("Too many sync wait commands"). Move excess waits onto
    preceding same-engine Drain carrier instructions."""
    m = json.loads(bir_bytes)
    for fn in m.get("functions", []):
        for bb in fn.get("blocks", []):
            out = []
            for inst in bb.get("instructions", []):
                si = inst.get("sync_info") or {}
                w = si.get("on_wait") or []
                if len(w) > max_waits:
                    head = w[: len(w) - max_waits]
                    si["on_wait"] = w[len(w) - max_waits:]
                    inst["sync_info"] = si
                    for k, wt in enumerate(head):
                        out.append({
                            "name": f"{inst['name']}_wsplit{k}",
                            "engine": inst["engine"],
                            "opcode": "Drain",
                            "ins": [], "outs": [],
                            "is_reset_sema": False,
                            "debug": inst.get("debug"),
                            "sync_info": {"on_wait": [wt], "on_update": []},
                        })
                out.append(inst)
            bb["instructions"] = out
    return json.dumps(m).encode()


def _dedup_ldweights(m):
    """Walrus is run with --enable-ldw-opt=false, so every Matmult comes
    with its own Ldweights (~110ns on the PE pipe). The PE retains the
    stationary operand across matmuls, so when consecutive matmuls share
    an identical weights AP the repeat Ldweights are pure overhead: drop
    them, preserving any semaphore waits/updates on a cheap Drain
    carrier. Only Matmults may sit between a kept Ldweights and its
    reusers; any other PE instruction resets the tracked signature."""
    for fn in m.get("functions", []):
        for bb in fn.get("blocks", []):
            out = []
            last_sig = None
            for inst in bb["instructions"]:
                if inst.get("engine") != "PE":
                    out.append(inst)
                    continue
                op = inst.get("opcode")
                if op == "Ldweights":
                    sig = json.dumps(
                        [inst.get("ins"), inst.get("perf_mode"),
                         inst.get("is_transpose"), inst.get("tile_position"),
                         inst.get("tile_size")],
                        sort_keys=True)
                    if sig == last_sig:
                        si = inst.get("sync_info") or {}
                        if si.get("on_wait") or si.get("on_update"):
                            out.append({
                                "name": f"{inst['name']}_ldwdrop",
                                "engine": "PE",
                                "opcode": "Drain",
                                "ins": [], "outs": [],
                                "is_reset_sema": False,
                                "debug": inst.get("debug"),
                                "sync_info": si,
                            })
                        continue
                    last_sig = sig
                    out.append(inst)
                elif op == "Matmult":
                    out.append(inst)
                else:
                    last_sig = None
                    out.append(inst)
            bb["instructions"] = out
    return m


def _install_birpatch(nc, dedup_ldw=True):
    orig = nc.to_json_bytes

    def patched():
        m = json.loads(orig())
        if dedup_ldw:
            m = _dedup_ldweights(m)
        return _split_multi_waits(json.dumps(m).encode())

    nc.to_json_bytes = patched


def _build(t_total=T, hbufs=4, hprec="f8f16", scores_after=3, tch=8,
           pairs=1, pairdt=None, pbufs=2, chunk_last=2, npe=16,
           dedup_ldw=0, quads=None):
    # dedup_ldw=1 deletes repeat Ldweights for identical stationaries —
    # MEASURED BROKEN on this toolchain (PE does not retain the
    # stationary without its paired Ldweights; outputs garbage). Keep 0.
    import concourse.bass as bass
    import concourse.mybir as mybir
    from concourse import tile
    from concourse.masks import make_identity

    f32 = mybir.dt.float32
    f32r = mybir.dt.float32r
    X = mybir.AxisListType.X
    AO = mybir.AluOpType

    # 'f8f16': rows 0..T/2-1 arrive as host-side error-diffused fp8e4m3
    # (the per-column quantization residuals telescope, so the T-sum sees
    # only the final sub-ulp carry — denom relmax 1.7e-3 on the real
    # inputs) and rows T/2.. as fp16. Stream drops 32->24MB; the fp8
    # tiles' pair-adds still emit fp16 pairs, so the PE side is
    # unchanged. n8 fp8 tiles must be even (2 per ring) for balance.
    hybrid8 = hprec == "f8f16"
    two_byte = hprec in ("f16", "bf16") or hybrid8
    hdt = {
        "f16": mybir.dt.float16,
        "bf16": mybir.dt.bfloat16,
        "f8f16": mybir.dt.float16,
        "f32r": f32r,
        "f32": f32,
    }[hprec]
    f8 = mybir.dt.float8e4
    if not two_byte:
        pairs = 0
    if hybrid8:
        pairs = 1
    if pairdt is None:
        pairdt = "f16" if hybrid8 else (hprec if two_byte else "f32r")
    # quads: second-level DVE tree on the fp16 tiles. MEASURED SLOWER
    # (~115us vs ~91): the in-place quad chains pair-adds -> quad-add ->
    # matmuls serially per tile, so the PE stalls on the DVE instead of
    # overlapping it, despite the lower total PE work. Keep 0.
    if quads is None:
        quads = 0
    pdt = {
        "f16": mybir.dt.float16,
        "bf16": mybir.dt.bfloat16,
        "f32r": f32r,
    }[pairdt]

    tile_t = 128 * tch
    ntiles = t_total // tile_t
    assert ntiles * tile_t == t_total
    ncp = tch // 2
    if pairs:
        assert 1 <= chunk_last <= ntiles and tch >= 4

    # pairs=0 fallback: PE-direct on some tiles, DVE-acc on the rest
    if pairs:
        pe_tiles = set(range(ntiles))
    elif npe >= ntiles:
        pe_tiles = set(range(ntiles))
    elif npe <= 0:
        pe_tiles = set()
    else:
        step = max(1, (ntiles - 1) // npe)
        pe_tiles = set()
        i = ntiles - 1
        while len(pe_tiles) < npe and i > 0:
            pe_tiles.add(i)
            i -= step
    have_dve = len(pe_tiles) < ntiles

    nc = bass.Bass()
    n8 = (ntiles // 2) if hybrid8 else 0   # fp8 tiles: first half, 2/ring
    t8 = n8 * tile_t
    if hybrid8:
        h8_d = nc.dram_tensor("h8", [t8, F], f8, kind="ExternalInput")
        h_d = nc.dram_tensor("h", [t_total - t8, F], hdt,
                             kind="ExternalInput")
    else:
        h8_d = None
        h_d = nc.dram_tensor("h", [t_total, F], hdt, kind="ExternalInput")
    hs_d = nc.dram_tensor("hs", [N, F], f32, kind="ExternalInput")
    s_d = nc.dram_tensor("s", [BL, H], f32, kind="ExternalInput")
    w_d = nc.dram_tensor("w", [H, H], f32, kind="ExternalInput")
    b_d = nc.dram_tensor("bias", [1, H], f32, kind="ExternalInput")
    out_d = nc.dram_tensor("out", [BL, H], f32, kind="ExternalOutput")

    with tile.TileContext(nc) as tc:
        with (
            tc.tile_pool(name="consts", bufs=1) as consts,
            tc.tile_pool(name="small", bufs=1) as small,
            tc.tile_pool(name="hpool", bufs=hbufs) as hpool,
            tc.tile_pool(name="ppool", bufs=pbufs) as ppool,
            tc.tile_pool(name="psum", bufs=1, space=bass.MemorySpace.PSUM) as psum,
            tc.tile_pool(name="psumb", bufs=1, space=bass.MemorySpace.PSUM) as psumb,
        ):
            # ---- first h-tile DMAs lead each ring's queue ----
            h_view = h_d[:].rearrange("(i p c) f -> i p c f", p=128, c=tch)
            h8_view = (h8_d[:].rearrange("(i p c) f -> i p c f", p=128, c=tch)
                       if hybrid8 else None)

            # fp8 tiles in stream slots 0,1,4,5 (interleaved with fp16):
            # the DVE's 1x fp8 pair-adds (~10.8us/tile) outrun the fp8
            # DMA cadence, so an fp8-first order lets the DVE lag 2
            # tiles and stall the rings via hpool WAR (~7us). Interleaving
            # lets it catch up during the cheap fp16 tiles. h8 rows stay
            # contiguous (one diffusion range); the T-sum is order-free.
            f8_slots = tuple(range(ntiles))
            if hybrid8:
                half_n = ntiles // 4
                f8_slots = tuple(
                    s for g in range(ntiles // 4)
                    for s in (4 * g, 4 * g + 1))[:n8]

            def issue_tile(i):
                eng = nc.sync if i % 2 == 0 else nc.scalar
                if hybrid8 and i in f8_slots:
                    ht = hpool.tile([128, tch, F], f8, tag="htile",
                                    name=f"ht8_{i}")
                    eng.dma_start(out=ht[:], in_=h8_view[f8_slots.index(i)])
                else:
                    i16 = (sum(1 for s in range(i) if s not in f8_slots)
                           if hybrid8 else i)
                    ht = hpool.tile([128, tch, F], hdt, tag="htile",
                                    name=f"ht_{i}")
                    if i >= ntiles - chunk_last:
                        _dma_chunks(nc, eng, ht, h_view, i16, tch)
                    else:
                        eng.dma_start(out=ht[:], in_=h_view[i16])
                return ht

            htiles = [issue_tile(i) for i in range(min(2, ntiles))]

            # ---- small loads, ring-balanced ----
            s_sb = small.tile([BL, H], f32)
            nc.scalar.dma_start(out=s_sb[:], in_=s_d[:])
            b_sb = small.tile([1, H], f32)
            nc.scalar.dma_start(out=b_sb[:], in_=b_d[:])
            # w split across rings to balance totals to ~0.01MB
            w_sb = small.tile([128, 2, H], f32)
            w_view = w_d[:].rearrange("(c p) d -> p c d", p=128)
            nc.sync.dma_start(out=w_sb[:, 0, :], in_=w_view[:, 0, :])
            nc.scalar.dma_start(out=w_sb[:, 1, :], in_=w_view[:, 1, :])
            hs_sb = small.tile([N, F], f32)
            half = F // 2
            nc.sync.dma_start(out=hs_sb[:, 0:half], in_=hs_d[:, 0:half])
            nc.scalar.dma_start(out=hs_sb[:, half:F], in_=hs_d[:, half:F])

            # ---- constants ----
            ident = consts.tile([128, 128], f32)
            make_identity(nc, ident[:])
            # E3[p, c, m] = 1.0 iff m == c ; E3[:, b, :] is the ones-column
            # selector landing batch b's column sums on PSUM partition b.
            e3 = consts.tile([128, BL, BL], f32)
            nc.gpsimd.memset(e3[:], 0.0)
            nc.gpsimd.affine_select(
                out=e3[:], in_=e3[:], compare_op=AO.not_equal, fill=1.0,
                base=0, pattern=[[-1, BL], [1, BL]], channel_multiplier=0,
            )
            ones1 = consts.tile([1, 128], f32)
            nc.gpsimd.memset(ones1[:], 1.0)
            ones8 = consts.tile([BL, 128], f32)
            nc.gpsimd.memset(ones8[:], 1.0)
            # ebd[b, b', h] = 1.0 iff b' == b  (block-diagonal placement mask)
            ebd = consts.tile([BL, BL, H], f32)
            nc.gpsimd.memset(ebd[:], 0.0)
            nc.gpsimd.affine_select(
                out=ebd[:], in_=ebd[:], compare_op=AO.not_equal, fill=1.0,
                base=0, pattern=[[-1, BL], [0, H]], channel_multiplier=1,
            )
            if pairs:
                e3p = consts.tile([128, BL, BL], pdt)
                nc.vector.tensor_copy(out=e3p[:], in_=e3[:])
                if hdt is pdt:
                    e3x = e3p
                else:
                    e3x = consts.tile([128, BL, BL], hdt)
                    nc.vector.tensor_copy(out=e3x[:], in_=e3[:])
                e3m = None
            else:
                if hdt is not f32:
                    e3x = consts.tile([128, BL, BL], hdt)
                    nc.vector.tensor_copy(out=e3x[:], in_=e3[:])
                else:
                    e3x = e3
                if have_dve and hdt not in (f32, f32r):
                    e3m = consts.tile([128, BL, BL], f32r)
                    nc.vector.tensor_copy(out=e3m[:], in_=e3[:])
                elif hdt is f32r:
                    e3m = e3x
                else:
                    e3m = e3

            def sw_path():
                # transposes: s [8,256] -> s_T [d,b]; W [h,d] -> W_T [d,h]
                s_t = small.tile([128, 2, BL], f32)
                for c in range(2):
                    pst = psum.tile([128, BL], f32, tag="tmp")
                    nc.tensor.transpose(
                        pst[:], s_sb[:, c * 128:(c + 1) * 128], ident[0:BL, 0:BL]
                    )
                    nc.vector.tensor_copy(out=s_t[:, c, :], in_=pst[:])
                w_t = small.tile([128, 2, H], f32)
                for c in range(2):
                    for hc in range(2):
                        ptw = psum.tile([128, 128], f32, tag="tmp")
                        nc.tensor.transpose(
                            ptw[:], w_sb[:, hc, c * 128:(c + 1) * 128], ident[:]
                        )
                        nc.vector.tensor_copy(
                            out=w_t[:, c, hc * 128:(hc + 1) * 128], in_=ptw[:]
                        )

                # sW = s @ W.T + b  -> [BL, H] (batch on partitions)
                ps_sw = psum.tile([BL, H], f32, tag="tmp")
                nc.tensor.matmul(ps_sw[:], s_t[:, 0, :], w_t[:, 0, :],
                                 start=True, stop=False)
                nc.tensor.matmul(ps_sw[:], s_t[:, 1, :], w_t[:, 1, :],
                                 start=False, stop=False)
                nc.tensor.matmul(ps_sw[:], ones1[0:1, 0:BL], b_sb[:],
                                 start=False, stop=True)
                sw_sb = small.tile([BL, H], f32)
                nc.vector.tensor_copy(out=sw_sb[:], in_=ps_sw[:])

                # sW placed block-diagonally: sw_bd[b, b', :] = sW[b]*[b'==b]
                # so ones8^T @ sw_bd broadcasts sW to all 128 partitions
                # with no DRAM bounce.
                sw_bd = small.tile([BL, BL, H], f32)
                nc.vector.tensor_mul(
                    out=sw_bd[:],
                    in0=sw_sb[:].unsqueeze(1).to_broadcast((BL, BL, H)),
                    in1=ebd[:],
                )
                return sw_sb, sw_bd[:].rearrange("b a h -> b (a h)")

            def scores_part1(sw_bd_flat):
                # broadcast sW to all 128 partitions (PE)
                ps_bc = psum.tile([128, F], f32, tag="big4")
                for c in range(4):
                    nc.tensor.matmul(
                        ps_bc[:, c * 512:(c + 1) * 512],
                        ones8[:], sw_bd_flat[:, c * 512:(c + 1) * 512],
                        start=True, stop=True,
                    )
                # scores_raw[n, b] = sum_h sW[b,h] * hs[n,b,h]
                prod = small.tile([N, F], f32)
                nc.vector.tensor_mul(out=prod[:], in0=hs_sb[:], in1=ps_bc[:])
                scores = small.tile([N, BL], f32)
                nc.vector.reduce_sum(
                    out=scores[:],
                    in_=prod[:].rearrange("n (b h) -> n b h", b=BL), axis=X,
                )
                # scoresE[:, b, :] is scores[:, b] placed in column b, zeros
                # elsewhere, so each matmul only lands on PSUM partition b.
                scores_e = small.tile([N, BL, BL], f32)
                nc.vector.tensor_mul(
                    out=scores_e[:],
                    in0=scores[:].unsqueeze(2).to_broadcast((N, BL, BL)),
                    in1=e3[:],
                )
                return scores_e

            def scores_part2(scores_e):
                ps_o = psum.tile([BL, H], f32, tag="cout")
                for bb in range(BL):
                    nc.tensor.matmul(
                        ps_o[:], scores_e[:, bb, :],
                        hs_sb[:, bb * H:(bb + 1) * H],
                        start=(bb == 0), stop=(bb == BL - 1),
                        skip_group_check=True,
                    )
                return ps_o

            # ---- the big stream: h_sum over T ----
            ps8 = psumb.tile([BL, H], f32)
            acc = (small.tile([128, F], f32 if hdt is f32 else f32r,
                              name="acc", tag="acc") if have_dve else None)
            first_mm = True
            first_dve = True
            last_mm_pos = max(pe_tiles) if pe_tiles else -1
            sw_sb = sw_bd_flat = None
            scores_e = None
            ps_o = None
            tail_start = ntiles - chunk_last if pairs else ntiles
            tail_ht = {}
            for i in range(ntiles):
                if i < len(htiles):
                    ht = htiles[i]
                else:
                    ht = issue_tile(i)
                if i >= tail_start:
                    # consumption emitted after the loop, interleaved
                    # across the rings' last tiles
                    tail_ht[i] = ht
                elif pairs:
                    pt = ppool.tile([128, ncp, F], pdt, tag="pair")
                    use_quads = quads and i >= n8
                    for cp in range(ncp):
                        c0, c1 = 2 * cp, 2 * cp + 1
                        nc.vector.tensor_add(
                            out=pt[:, cp, :],
                            in0=ht[:, c0, :], in1=ht[:, c1, :])
                        if not use_quads:
                            for bb in range(BL):
                                nc.tensor.matmul(
                                    ps8[:], e3p[:, bb, :],
                                    pt[:, cp, bb * H:(bb + 1) * H],
                                    start=first_mm, stop=False,
                                    skip_group_check=True,
                                )
                                first_mm = False
                    if use_quads:
                        # quads folded in place: pt[2cq] += pt[2cq+1]
                        for cq in range(ncp // 2):
                            nc.vector.tensor_add(
                                out=pt[:, 2 * cq, :],
                                in0=pt[:, 2 * cq, :], in1=pt[:, 2 * cq + 1, :])
                            for bb in range(BL):
                                nc.tensor.matmul(
                                    ps8[:], e3p[:, bb, :],
                                    pt[:, 2 * cq, bb * H:(bb + 1) * H],
                                    start=first_mm, stop=False,
                                    skip_group_check=True,
                                )
                                first_mm = False
                elif i not in pe_tiles:
                    for c in range(tch):
                        if first_dve:
                            nc.vector.tensor_copy(out=acc[:], in_=ht[:, c, :])
                            first_dve = False
                        else:
                            nc.vector.tensor_add(
                                out=acc[:], in0=acc[:], in1=ht[:, c, :])
                else:
                    for bb in range(BL):
                        for c in range(tch):
                            stop = (not have_dve and i == last_mm_pos
                                    and c == tch - 1 and bb == BL - 1)
                            nc.tensor.matmul(
                                ps8[:], e3x[:, bb, :],
                                ht[:, c, bb * H:(bb + 1) * H],
                                start=first_mm, stop=stop,
                                skip_group_check=True,
                            )
                            first_mm = False
                if i == min(1, ntiles - 1):
                    sw_sb, sw_bd_flat = sw_path()
                if i == scores_after:
                    scores_e = scores_part1(sw_bd_flat)
                if i == scores_after + 2:
                    ps_o = scores_part2(scores_e)
            if scores_e is None:
                scores_e = scores_part1(sw_bd_flat)
            if ps_o is None:
                ps_o = scores_part2(scores_e)

            # ---- interleaved tail: the rings' last tiles trail their
            # chunk DMAs together (the DVE and PE are in-order, so a
            # per-tile emission would serialize tile 7 behind tile 6's
            # final chunk). The final two chunks skip the pair stage and
            # go straight to the PE - after the very last half-chunk
            # lands only 4 matmuls + the epilogue remain (~3.5us).
            tt = sorted(tail_ht)
            if tt:
                pts = {i: ppool.tile([128, ncp, F], pdt, tag="pair",
                                     name=f"ptail{i}")
                       for i in tt}
                for cp in range(ncp - 1):
                    c0, c1 = 2 * cp, 2 * cp + 1
                    for i in tt:
                        ht = tail_ht[i]
                        nc.vector.tensor_add(
                            out=pts[i][:, cp, :],
                            in0=ht[:, c0, :], in1=ht[:, c1, :])
                        if quads and cp == 1:
                            # fold pairs 0,1 (chunks 0-3, landed early)
                            # into one in-place quad: 8 matmuls not 16
                            nc.vector.tensor_add(
                                out=pts[i][:, 0, :],
                                in0=pts[i][:, 0, :], in1=pts[i][:, 1, :])
                            src, scp = pts[i], 0
                        elif quads and cp == 0:
                            continue   # consumed by the quad at cp==1
                        else:
                            src, scp = pts[i], cp
                        for bb in range(BL):
                            nc.tensor.matmul(
                                ps8[:], e3p[:, bb, :],
                                src[:, scp, bb * H:(bb + 1) * H],
                                start=first_mm, stop=False,
                                skip_group_check=True,
                            )
                            first_mm = False
                for i in tt:  # raw chunk tch-2
                    for bb in range(BL):
                        nc.tensor.matmul(
                            ps8[:], e3x[:, bb, :],
                            tail_ht[i][:, tch - 2, bb * H:(bb + 1) * H],
                            start=first_mm, stop=False,
                            skip_group_check=True,
                        )
                        first_mm = False
                # raw final chunk, quarter by quarter (each quarter's
                # columns are exactly batches 2q, 2q+1); the last quarter
                # arrives as two batch-aligned eighths, so bb=6's matmuls
                # run while bb=7's 64KB is still in flight
                for q in range(3):
                    for i in tt:
                        for bb in (2 * q, 2 * q + 1):
                            nc.tensor.matmul(
                                ps8[:], e3x[:, bb, :],
                                tail_ht[i][:, tch - 1, bb * H:(bb + 1) * H],
                                start=first_mm, stop=False,
                                skip_group_check=True,
                            )
                            first_mm = False
                for bb in (BL - 2, BL - 1):
                    for i in tt:
                        stop = (bb == BL - 1 and i == tt[-1])
                        nc.tensor.matmul(
                            ps8[:], e3x[:, bb, :],
                            tail_ht[i][:, tch - 1, bb * H:(bb + 1) * H],
                            start=first_mm, stop=stop,
                            skip_group_check=True,
                        )
                        first_mm = False

            # land the DVE accumulator's per-batch column sums on ps8
            if not pairs and have_dve:
                for bb in range(BL):
                    nc.tensor.matmul(
                        ps8[:], e3m[:, bb, :],
                        acc[:, bb * H:(bb + 1) * H],
                        start=first_mm, stop=(bb == BL - 1),
                        skip_group_check=True,
                    )
                    first_mm = False

            # ---- denom, reciprocal, final scale, store ----
            denq = small.tile([BL, H], f32)
            den = small.tile([BL, 1], f32)
            nc.vector.tensor_mul(out=denq[:], in0=sw_sb[:], in1=ps8[:])
            nc.vector.reduce_sum(out=den[:], in_=denq[:], axis=X)
            inv = small.tile([BL, 1], f32)
            nc.vector.reciprocal(out=inv[:], in_=den[:])
            c_fin = small.tile([BL, H], f32)
            nc.vector.tensor_scalar_mul(out=c_fin[:], in0=ps_o[:], scalar1=inv[:])
            # SP's DGE has the lowest issue+start latency (565+650 ns)
            nc.sync.dma_start(out=out_d[:], in_=c_fin[:])

    _install_birpatch(nc, dedup_ldw=bool(dedup_ldw))
    return nc


def _dma_chunks(nc, eng, ht, h_view, i, tch):
    """Chunked tile DMA (512KB chunks; the final chunk split in quarters,
    the last quarter further into batch-aligned eighths — so after the
    very last 64KB lands only one matmul per tile remains)."""
    Fq = F // 4
    Fe = F // 8
    for c in range(tch):
        if c == tch - 1:
            for q in range(3):
                eng.dma_start(out=ht[:, c, q * Fq:(q + 1) * Fq],
                              in_=h_view[i][:, c, q * Fq:(q + 1) * Fq])
            for e in (6, 7):
                eng.dma_start(out=ht[:, c, e * Fe:(e + 1) * Fe],
                              in_=h_view[i][:, c, e * Fe:(e + 1) * Fe])
        else:
            eng.dma_start(out=ht[:, c, :], in_=h_view[i][:, c, :])


def _get_nc(**kw):
    key = tuple(sorted(kw.items()))
    if key not in _CACHE:
        _CACHE[key] = _build(**kw)
    return _CACHE[key]


def _np_hdt(hprec):
    if hprec == "f16":
        return np.float16
    if hprec == "bf16":
        import ml_dtypes
        return ml_dtypes.bfloat16
    return np.float32


def _diffuse_fp8(h32):
    """Error-diffused fp8e4m3 quantization along t, per (b, h) column:
    q_t = fp8(x_t + e_{t-1}), e_t = the residual. Sum(q) = Sum(x) - e_T,
    so the device's T-sum sees only the final sub-ulp carry per column
    instead of sqrt(T)-accumulated rounding noise."""
    import ml_dtypes
    f8 = ml_dtypes.float8_e4m3   # matches mybir.dt.np(float8e4)
    q = np.empty(h32.shape, dtype=f8)
    e = np.zeros(h32.shape[1:], np.float32)
    for t in range(h32.shape[0]):
        x = h32[t] + e
        qt = x.astype(f8)
        q[t] = qt
        e = x - qt.astype(np.float32)
    return q


def _shard_inputs(s_before, h_sliced, h, W, b, t_total=T, hprec="f8f16"):
    in_maps = []
    if hprec == "f8f16":
        t8 = t_total // 2
        q8 = _diffuse_fp8(h[:t8].astype(np.float32))
        h16 = h[t8:t_total].astype(np.float16)
        for i in range(NCORES):
            sl = slice(i * BL, (i + 1) * BL)
            in_maps.append({
                "h8": np.ascontiguousarray(q8[:, sl, :]).reshape(t8, F),
                "h": np.ascontiguousarray(h16[:, sl, :]).reshape(
                    t_total - t8, F),
                "hs": np.ascontiguousarray(h_sliced[:, sl, :]).reshape(N, F),
                "s": np.ascontiguousarray(s_before[0, sl, :]),
                "w": np.ascontiguousarray(W),
                "bias": np.ascontiguousarray(b).reshape(1, H),
            })
        return in_maps
    ndt = _np_hdt(hprec)
    for i in range(NCORES):
        sl = slice(i * BL, (i + 1) * BL)
        in_maps.append({
            "h": np.ascontiguousarray(
                h[:t_total, sl, :].astype(ndt)).reshape(t_total, F),
            "hs": np.ascontiguousarray(h_sliced[:, sl, :]).reshape(N, F),
            "s": np.ascontiguousarray(s_before[0, sl, :]),
            "w": np.ascontiguousarray(W),
            "bias": np.ascontiguousarray(b).reshape(1, H),
        })
    return in_maps


def _run(s_before, h_sliced, h, W, b, trace=False, **build_kw):
    from concourse.bass_utils import run_bass_kernel_spmd

    nc = _get_nc(**build_kw)
    in_maps = _shard_inputs(s_before, h_sliced, h, W, b,
                            t_total=build_kw.get("t_total", T),
                            hprec=build_kw.get("hprec", "f8f16"))
    bkr = run_bass_kernel_spmd(nc, in_maps, list(range(NCORES)), trace=trace)
    out = np.concatenate([bkr.results[i]["out"] for i in range(NCORES)], axis=0)
    return out, bkr


def kernel(s_before, h_sliced, h, W, b):
    out, _ = _run(
        np.asarray(s_before), np.asarray(h_sliced), np.asarray(h),
        np.asarray(W), np.asarray(b),
    )
    return out


# revision 61
# speedup vs baseline: 1.0784x; 1.0784x over previous
"""Trainium2 Bass kernel for nn_Attention_35708358099413.

Reference computation (T=8192, B=64, H=256, N=128):
    sW     = s_before @ W.T + b                      # [1,B,H]
    denom  = einsum('obd,tbd->ob', sW, h)            # [1,B] (sum over T and H)
    scores = einsum('obd,nbd->obn', sW, h_sliced) / denom
    c_t    = (scores.T * h_sliced).sum(0)            # [B,H]

Strategy: pure data-parallel over batch. 8 cores x 8 batches each; no
collectives. Per core the dominant work is h_sum[b,d] = sum_t h[t,b,d].

Precision split (load-bearing): h's ONLY use is the per-batch scalar
denom (min |denom| = 210 on the real inputs), whose error scales each
batch's output UNIFORMLY - so h tolerates fp16 (host-side cast halves
the stream to 32MB/core; measured 2.1e-3 output relmax). The c_t chain
(h_sliced, scores, ps_o) must stay strictly fp32: min |expected
output| = 4e-5 with the harness's elementwise rel-err metric, so even
1e-5-level ABSOLUTE noise there fails.

Per-core pipeline (hprec='f16', pairs=1 default):
  - h [T, 8*256] fp16 viewed [8, 128, 8, 2048]; 4MB tiles (32KB per
    partition, the efficient descriptor size) stream on the two HWDGE
    rings (sync/scalar alternating). The last tile of EACH ring is
    chunked (512KB) so its consumers trail chunk-wise. Ring loads are
    balanced: w + half of hs ride sync, s/b + other hs half scalar
    (16.75 vs 16.51 MB).
  - Walrus emits an LDWEIGHTS per matmul (--enable-ldw-opt=false), so
    PE-only reduction costs 64x(124+107)ns = 15us/tile > 12us DMA.
    Instead the DVE adds chunk PAIRS in fp16 first (all-2-byte SBUF
    operands hit the DVE 4x mode, ~0.6us per [128,2048] add), then the
    PE consumes pairs: 32 matmuls/tile (~7.4us). Each h element takes
    exactly one extra fp16 rounding: denom relmax 3.3e-3 (simulated on
    the real inputs), 6x inside the 2e-2 gate.
  - PE matmul trick: lhsT = e3[:, b, :] (ones in column b) lands batch
    b's column sums on PSUM partition b, accumulating into one [8, 256]
    PSUM tile across all tiles. cp-outer ordering means the tail trails
    chunk-wise (LDW cost is per-matmul anyway, so ordering is free);
    the rings' final chunks split into batch-aligned quarters/eighths so
    only 2 matmuls follow the very last 64KB.
  - sW = s @ W.T + b on PE from on-chip transposes of s and W, emitted
    mid-stream. sW is broadcast to all 128 partitions by placing it
    block-diagonally ([8, 8*256], DVE mask multiply) and multiplying by
    ones8 on PE - no DRAM bounce.
  - scores_raw[n,b] = rowwise reduce of (h_sliced * bcast_sW) on DVE
    (tile 3); c_raw[b,:] = scores^T @ h_sliced on PE via masked score
    columns (tile 5, so its fp32 matmuls never sit in the tail).
    denom[b] = <sW[b], h_sum[b]> and the 1/denom scale fold in at the
    very end (~2us tail).

hprec='f32r' pairs=0 reproduces the previous all-fp32 kernel (~188us);
f16+pairs measures ~102us (fast draw) against a ~92.5us stream floor
(360 GB/s per-core DMA cap = 2 rings x 8 engines x 22.5 GB/s; ~7us
framework start + ~3.5us tail are the rest). Exec is bimodal: slow
draws ~118us trace to an 11us initial cross-engine barrier
(sibling-core launch skew), not to anything in this program.

Negative results (do not retry): deleting "redundant" Ldweights via a
BIR post-pass compiles but yields garbage on hardware - the PE does
not retain the stationary operand without each matmul's paired
Ldweights (and PE time wasn't the bottleneck anyway).
tensor_tensor_reduce crashes walrus codegen ("ISA wrong length").
"""

import json

import numpy as np

T, B, H, N = 8192, 64, 256, 128
NCORES = 8
BL = B // NCORES          # 8 batches per core
F = BL * H                # 2048

_CACHE = {}


def _split_multi_waits(bir_bytes, max_waits=1):
    """Walrus in some containers rejects instructions carrying more than
    one sem wait ("Too many sync wait commands"). Move excess waits onto
    preceding same-engine Drain carrier instructions."""
    m = json.loads(bir_bytes)
    for fn in m.get("functions", []):
        for bb in fn.get("blocks", []):
            out = []
            for inst in bb.get("instructions", []):
                si = inst.get("sync_info") or {}
                w = si.get("on_wait") or []
                if len(w) > max_waits:
                    head = w[: len(w) - max_waits]
                    si["on_wait"] = w[len(w) - max_waits:]
                    inst["sync_info"] = si
                    for k, wt in enumerate(head):
                        out.append({
                            "name": f"{inst['name']}_wsplit{k}",
                            "engine": inst["engine"],
                            "opcode": "Drain",
                            "ins": [], "outs": [],
                            "is_reset_sema": False,
                            "debug": inst.get("debug"),
                            "sync_info": {"on_wait": [wt], "on_update": []},
                        })
                out.append(inst)
            bb["instructions"] = out
    return json.dumps(m).encode()


def _dedup_ldweights(m):
    """Walrus is run with --enable-ldw-opt=false, so every Matmult comes
    with its own Ldweights (~110ns on the PE pipe). The PE retains the
    stationary operand across matmuls, so when consecutive matmuls share
    an identical weights AP the repeat Ldweights are pure overhead: drop
    them, preserving any semaphore waits/updates on a cheap Drain
    carrier. Only Matmults may sit between a kept Ldweights and its
    reusers; any other PE instruction resets the tracked signature."""
    for fn in m.get("functions", []):
        for bb in fn.get("blocks", []):
            out = []
            last_sig = None
            for inst in bb["instructions"]:
                if inst.get("engine") != "PE":
                    out.append(inst)
                    continue
                op = inst.get("opcode")
                if op == "Ldweights":
                    sig = json.dumps(
                        [inst.get("ins"), inst.get("perf_mode"),
                         inst.get("is_transpose"), inst.get("tile_position"),
                         inst.get("tile_size")],
                        sort_keys=True)
                    if sig == last_sig:
                        si = inst.get("sync_info") or {}
                        if si.get("on_wait") or si.get("on_update"):
                            out.append({
                                "name": f"{inst['name']}_ldwdrop",
                                "engine": "PE",
                                "opcode": "Drain",
                                "ins": [], "outs": [],
                                "is_reset_sema": False,
                                "debug": inst.get("debug"),
                                "sync_info": si,
                            })
                        continue
                    last_sig = sig
                    out.append(inst)
                elif op == "Matmult":
                    out.append(inst)
                else:
                    last_sig = None
                    out.append(inst)
            bb["instructions"] = out
    return m


def _install_birpatch(nc, dedup_ldw=True):
    orig = nc.to_json_bytes

    def patched():
        m = json.loads(orig())
        if dedup_ldw:
            m = _dedup_ldweights(m)
        return _split_multi_waits(json.dumps(m).encode())

    nc.to_json_bytes = patched


def _build(t_total=T, hbufs=4, hprec="f8f16", scores_after=3, tch=8,
           pairs=1, pairdt=None, pbufs=2, chunk_last=2, npe=16,
           dedup_ldw=0, quads=None):
    # dedup_ldw=1 deletes repeat Ldweights for identical stationaries —
    # MEASURED BROKEN on this toolchain (PE does not retain the
    # stationary without its paired Ldweights; outputs garbage). Keep 0.
    import concourse.bass as bass
    import concourse.mybir as mybir
    from concourse import tile
    from concourse.masks import make_identity

    f32 = mybir.dt.float32
    f32r = mybir.dt.float32r
    X = mybir.AxisListType.X
    AO = mybir.AluOpType

    # 'f8f16': rows 0..T/2-1 arrive as host-side error-diffused fp8e4m3
    # (the per-column quantization residuals telescope, so the T-sum sees
    # only the final sub-ulp carry — denom relmax 1.7e-3 on the real
    # inputs) and rows T/2.. as fp16. Stream drops 32->24MB; the fp8
    # tiles' pair-adds still emit fp16 pairs, so the PE side is
    # unchanged. n8 fp8 tiles must be even (2 per ring) for balance.
    hybrid8 = hprec == "f8f16"
    two_byte = hprec in ("f16", "bf16") or hybrid8
    hdt = {
        "f16": mybir.dt.float16,
        "bf16": mybir.dt.bfloat16,
        "f8f16": mybir.dt.float16,
        "f32r": f32r,
        "f32": f32,
    }[hprec]
    f8 = mybir.dt.float8e4
    if not two_byte:
        pairs = 0
    if hybrid8:
        pairs = 1
    if pairdt is None:
        pairdt = "f16" if hybrid8 else (hprec if two_byte else "f32r")
    # quads: second-level DVE tree on the fp16 tiles. MEASURED SLOWER
    # (~115us vs ~91): the in-place quad chains pair-adds -> quad-add ->
    # matmuls serially per tile, so the PE stalls on the DVE instead of
    # overlapping it, despite the lower total PE work. Keep 0.
    if quads is None:
        quads = 0
    pdt = {
        "f16": mybir.dt.float16,
        "bf16": mybir.dt.bfloat16,
        "f32r": f32r,
    }[pairdt]

    tile_t = 128 * tch
    ntiles = t_total // tile_t
    assert ntiles * tile_t == t_total
    ncp = tch // 2
    if pairs:
        assert 1 <= chunk_last <= ntiles and tch >= 4

    # pairs=0 fallback: PE-direct on some tiles, DVE-acc on the rest
    if pairs:
        pe_tiles = set(range(ntiles))
    elif npe >= ntiles:
        pe_tiles = set(range(ntiles))
    elif npe <= 0:
        pe_tiles = set()
    else:
        step = max(1, (ntiles - 1) // npe)
        pe_tiles = set()
        i = ntiles - 1
        while len(pe_tiles) < npe and i > 0:
            pe_tiles.add(i)
            i -= step
    have_dve = len(pe_tiles) < ntiles

    nc = bass.Bass()
    n8 = (ntiles // 2) if hybrid8 else 0   # fp8 tiles: first half, 2/ring
    t8 = n8 * tile_t
    if hybrid8:
        h8_d = nc.dram_tensor("h8", [t8, F], f8, kind="ExternalInput")
        h_d = nc.dram_tensor("h", [t_total - t8, F], hdt,
                             kind="ExternalInput")
    else:
        h8_d = None
        h_d = nc.dram_tensor("h", [t_total, F], hdt, kind="ExternalInput")
    hs_d = nc.dram_tensor("hs", [N, F], f32, kind="ExternalInput")
    s_d = nc.dram_tensor("s", [BL, H], f32, kind="ExternalInput")
    w_d = nc.dram_tensor("w", [H, H], f32, kind="ExternalInput")
    b_d = nc.dram_tensor("bias", [1, H], f32, kind="ExternalInput")
    out_d = nc.dram_tensor("out", [BL, H], f32, kind="ExternalOutput")

    with tile.TileContext(nc) as tc:
        with (
            tc.tile_pool(name="consts", bufs=1) as consts,
            tc.tile_pool(name="small", bufs=1) as small,
            tc.tile_pool(name="hpool", bufs=hbufs) as hpool,
            tc.tile_pool(name="ppool", bufs=pbufs) as ppool,
            tc.tile_pool(name="psum", bufs=1, space=bass.MemorySpace.PSUM) as psum,
            tc.tile_pool(name="psumb", bufs=1, space=bass.MemorySpace.PSUM) as psumb,
        ):
            # ---- first h-tile DMAs lead each ring's queue ----
            h_view = h_d[:].rearrange("(i p c) f -> i p c f", p=128, c=tch)
            h8_view = (h8_d[:].rearrange("(i p c) f -> i p c f", p=128, c=tch)
                       if hybrid8 else None)

            def issue_tile(i):
                eng = nc.sync if i % 2 == 0 else nc.scalar
                if i < n8:
                    ht = hpool.tile([128, tch, F], f8, tag="htile",
                                    name=f"ht8_{i}")
                    eng.dma_start(out=ht[:], in_=h8_view[i])
                else:
                    ht = hpool.tile([128, tch, F], hdt, tag="htile",
                                    name=f"ht_{i}")
                    if i >= ntiles - chunk_last:
                        _dma_chunks(nc, eng, ht, h_view, i - n8, tch)
                    else:
                        eng.dma_start(out=ht[:], in_=h_view[i - n8])
                return ht

            htiles = [issue_tile(i) for i in range(min(2, ntiles))]

            # ---- small loads, ring-balanced ----
            s_sb = small.tile([BL, H], f32)
            nc.scalar.dma_start(out=s_sb[:], in_=s_d[:])
            b_sb = small.tile([1, H], f32)
            nc.scalar.dma_start(out=b_sb[:], in_=b_d[:])
            # w split across rings to balance totals to ~0.01MB
            w_sb = small.tile([128, 2, H], f32)
            w_view = w_d[:].rearrange("(c p) d -> p c d", p=128)
            nc.sync.dma_start(out=w_sb[:, 0, :], in_=w_view[:, 0, :])
            nc.scalar.dma_start(out=w_sb[:, 1, :], in_=w_view[:, 1, :])
            hs_sb = small.tile([N, F], f32)
            half = F // 2
            nc.sync.dma_start(out=hs_sb[:, 0:half], in_=hs_d[:, 0:half])
            nc.scalar.dma_start(out=hs_sb[:, half:F], in_=hs_d[:, half:F])

            # ---- constants ----
            ident = consts.tile([128, 128], f32)
            make_identity(nc, ident[:])
            # E3[p, c, m] = 1.0 iff m == c ; E3[:, b, :] is the ones-column
            # selector landing batch b's column sums on PSUM partition b.
            e3 = consts.tile([128, BL, BL], f32)
            nc.gpsimd.memset(e3[:], 0.0)
            nc.gpsimd.affine_select(
                out=e3[:], in_=e3[:], compare_op=AO.not_equal, fill=1.0,
                base=0, pattern=[[-1, BL], [1, BL]], channel_multiplier=0,
            )
            ones1 = consts.tile([1, 128], f32)
            nc.gpsimd.memset(ones1[:], 1.0)
            ones8 = consts.tile([BL, 128], f32)
            nc.gpsimd.memset(ones8[:], 1.0)
            # ebd[b, b', h] = 1.0 iff b' == b  (block-diagonal placement mask)
            ebd = consts.tile([BL, BL, H], f32)
            nc.gpsimd.memset(ebd[:], 0.0)
            nc.gpsimd.affine_select(
                out=ebd[:], in_=ebd[:], compare_op=AO.not_equal, fill=1.0,
                base=0, pattern=[[-1, BL], [0, H]], channel_multiplier=1,
            )
            if pairs:
                e3p = consts.tile([128, BL, BL], pdt)
                nc.vector.tensor_copy(out=e3p[:], in_=e3[:])
                if hdt is pdt:
                    e3x = e3p
                else:
                    e3x = consts.tile([128, BL, BL], hdt)
                    nc.vector.tensor_copy(out=e3x[:], in_=e3[:])
                e3m = None
            else:
                if hdt is not f32:
                    e3x = consts.tile([128, BL, BL], hdt)
                    nc.vector.tensor_copy(out=e3x[:], in_=e3[:])
                else:
                    e3x = e3
                if have_dve and hdt not in (f32, f32r):
                    e3m = consts.tile([128, BL, BL], f32r)
                    nc.vector.tensor_copy(out=e3m[:], in_=e3[:])
                elif hdt is f32r:
                    e3m = e3x
                else:
                    e3m = e3

            def sw_path():
                # transposes: s [8,256] -> s_T [d,b]; W [h,d] -> W_T [d,h]
                s_t = small.tile([128, 2, BL], f32)
                for c in range(2):
                    pst = psum.tile([128, BL], f32, tag="tmp")
                    nc.tensor.transpose(
                        pst[:], s_sb[:, c * 128:(c + 1) * 128], ident[0:BL, 0:BL]
                    )
                    nc.vector.tensor_copy(out=s_t[:, c, :], in_=pst[:])
                w_t = small.tile([128, 2, H], f32)
                for c in range(2):
                    for hc in range(2):
                        ptw = psum.tile([128, 128], f32, tag="tmp")
                        nc.tensor.transpose(
                            ptw[:], w_sb[:, hc, c * 128:(c + 1) * 128], ident[:]
                        )
                        nc.vector.tensor_copy(
                            out=w_t[:, c, hc * 128:(hc + 1) * 128], in_=ptw[:]
                        )

                # sW = s @ W.T + b  -> [BL, H] (batch on partitions)
                ps_sw = psum.tile([BL, H], f32, tag="tmp")
                nc.tensor.matmul(ps_sw[:], s_t[:, 0, :], w_t[:, 0, :],
                                 start=True, stop=False)
                nc.tensor.matmul(ps_sw[:], s_t[:, 1, :], w_t[:, 1, :],
                                 start=False, stop=False)
                nc.tensor.matmul(ps_sw[:], ones1[0:1, 0:BL], b_sb[:],
                                 start=False, stop=True)
                sw_sb = small.tile([BL, H], f32)
                nc.vector.tensor_copy(out=sw_sb[:], in_=ps_sw[:])

                # sW placed block-diagonally: sw_bd[b, b', :] = sW[b]*[b'==b]
                # so ones8^T @ sw_bd broadcasts sW to all 128 partitions
                # with no DRAM bounce.
                sw_bd = small.tile([BL, BL, H], f32)
                nc.vector.tensor_mul(
                    out=sw_bd[:],
                    in0=sw_sb[:].unsqueeze(1).to_broadcast((BL, BL, H)),
                    in1=ebd[:],
                )
                return sw_sb, sw_bd[:].rearrange("b a h -> b (a h)")

            def scores_part1(sw_bd_flat):
                # broadcast sW to all 128 partitions (PE)
                ps_bc = psum.tile([128, F], f32, tag="big4")
                for c in range(4):
                    nc.tensor.matmul(
                        ps_bc[:, c * 512:(c + 1) * 512],
                        ones8[:], sw_bd_flat[:, c * 512:(c + 1) * 512],
                        start=True, stop=True,
                    )
                # scores_raw[n, b] = sum_h sW[b,h] * hs[n,b,h]
                prod = small.tile([N, F], f32)
                nc.vector.tensor_mul(out=prod[:], in0=hs_sb[:], in1=ps_bc[:])
                scores = small.tile([N, BL], f32)
                nc.vector.reduce_sum(
                    out=scores[:],
                    in_=prod[:].rearrange("n (b h) -> n b h", b=BL), axis=X,
                )
                # scoresE[:, b, :] is scores[:, b] placed in column b, zeros
                # elsewhere, so each matmul only lands on PSUM partition b.
                scores_e = small.tile([N, BL, BL], f32)
                nc.vector.tensor_mul(
                    out=scores_e[:],
                    in0=scores[:].unsqueeze(2).to_broadcast((N, BL, BL)),
                    in1=e3[:],
                )
                return scores_e

            def scores_part2(scores_e):
                ps_o = psum.tile([BL, H], f32, tag="cout")
                for bb in range(BL):
                    nc.tensor.matmul(
                        ps_o[:], scores_e[:, bb, :],
                        hs_sb[:, bb * H:(bb + 1) * H],
                        start=(bb == 0), stop=(bb == BL - 1),
                        skip_group_check=True,
                    )
                return ps_o

            # ---- the big stream: h_sum over T ----
            ps8 = psumb.tile([BL, H], f32)
            acc = (small.tile([128, F], f32 if hdt is f32 else f32r,
                              name="acc", tag="acc") if have_dve else None)
            first_mm = True
            first_dve = True
            last_mm_pos = max(pe_tiles) if pe_tiles else -1
            sw_sb = sw_bd_flat = None
            scores_e = None
            ps_o = None
            tail_start = ntiles - chunk_last if pairs else ntiles
            tail_ht = {}
            for i in range(ntiles):
                if i < len(htiles):
                    ht = htiles[i]
                else:
                    ht = issue_tile(i)
                if i >= tail_start:
                    # consumption emitted after the loop, interleaved
                    # across the rings' last tiles
                    tail_ht[i] = ht
                elif pairs:
                    pt = ppool.tile([128, ncp, F], pdt, tag="pair")
                    use_quads = quads and i >= n8
                    for cp in range(ncp):
                        c0, c1 = 2 * cp, 2 * cp + 1
                        nc.vector.tensor_add(
                            out=pt[:, cp, :],
                            in0=ht[:, c0, :], in1=ht[:, c1, :])
                        if not use_quads:
                            for bb in range(BL):
                                nc.tensor.matmul(
                                    ps8[:], e3p[:, bb, :],
                                    pt[:, cp, bb * H:(bb + 1) * H],
                                    start=first_mm, stop=False,
                                    skip_group_check=True,
                                )
                                first_mm = False
                    if use_quads:
                        # quads folded in place: pt[2cq] += pt[2cq+1]
                        for cq in range(ncp // 2):
                            nc.vector.tensor_add(
                                out=pt[:, 2 * cq, :],
                                in0=pt[:, 2 * cq, :], in1=pt[:, 2 * cq + 1, :])
                            for bb in range(BL):
                                nc.tensor.matmul(
                                    ps8[:], e3p[:, bb, :],
                                    pt[:, 2 * cq, bb * H:(bb + 1) * H],
                                    start=first_mm, stop=False,
                                    skip_group_check=True,
                                )
                                first_mm = False
                elif i not in pe_tiles:
                    for c in range(tch):
                        if first_dve:
                            nc.vector.tensor_copy(out=acc[:], in_=ht[:, c, :])
                            first_dve = False
                        else:
                            nc.vector.tensor_add(
                                out=acc[:], in0=acc[:], in1=ht[:, c, :])
                else:
                    for bb in range(BL):
                        for c in range(tch):
                            stop = (not have_dve and i == last_mm_pos
                                    and c == tch - 1 and bb == BL - 1)
                            nc.tensor.matmul(
                                ps8[:], e3x[:, bb, :],
                                ht[:, c, bb * H:(bb + 1) * H],
                                start=first_mm, stop=stop,
                                skip_group_check=True,
                            )
                            first_mm = False
                if i == min(1, ntiles - 1):
                    sw_sb, sw_bd_flat = sw_path()
                if i == scores_after:
                    scores_e = scores_part1(sw_bd_flat)
                if i == scores_after + 2:
                    ps_o = scores_part2(scores_e)
            if scores_e is None:
                scores_e = scores_part1(sw_bd_flat)
            if ps_o is None:
                ps_o = scores_part2(scores_e)

            # ---- interleaved tail: the rings' last tiles trail their
            # chunk DMAs together (the DVE and PE are in-order, so a
            # per-tile emission would serialize tile 7 behind tile 6's
            # final chunk). The final two chunks skip the pair stage and
            # go straight to the PE - after the very last half-chunk
            # lands only 4 matmuls + the epilogue remain (~3.5us).
            tt = sorted(tail_ht)
            if tt:
                pts = {i: ppool.tile([128, ncp, F], pdt, tag="pair",
                                     name=f"ptail{i}")
                       for i in tt}
                for cp in range(ncp - 1):
                    c0, c1 = 2 * cp, 2 * cp + 1
                    for i in tt:
                        ht = tail_ht[i]
                        nc.vector.tensor_add(
                            out=pts[i][:, cp, :],
                            in0=ht[:, c0, :], in1=ht[:, c1, :])
                        if quads and cp == 1:
                            # fold pairs 0,1 (chunks 0-3, landed early)
                            # into one in-place quad: 8 matmuls not 16
                            nc.vector.tensor_add(
                                out=pts[i][:, 0, :],
                                in0=pts[i][:, 0, :], in1=pts[i][:, 1, :])
                            src, scp = pts[i], 0
                        elif quads and cp == 0:
                            continue   # consumed by the quad at cp==1
                        else:
                            src, scp = pts[i], cp
                        for bb in range(BL):
                            nc.tensor.matmul(
                                ps8[:], e3p[:, bb, :],
                                src[:, scp, bb * H:(bb + 1) * H],
                                start=first_mm, stop=False,
                                skip_group_check=True,
                            )
                            first_mm = False
                for i in tt:  # raw chunk tch-2
                    for bb in range(BL):
                        nc.tensor.matmul(
                            ps8[:], e3x[:, bb, :],
                            tail_ht[i][:, tch - 2, bb * H:(bb + 1) * H],
                            start=first_mm, stop=False,
                            skip_group_check=True,
                        )
                        first_mm = False
                # raw final chunk, quarter by quarter (each quarter's
                # columns are exactly batches 2q, 2q+1); the last quarter
                # arrives as two batch-aligned eighths, so bb=6's matmuls
                # run while bb=7's 64KB is still in flight
                for q in range(3):
                    for i in tt:
                        for bb in (2 * q, 2 * q + 1):
                            nc.tensor.matmul(
                                ps8[:], e3x[:, bb, :],
                                tail_ht[i][:, tch - 1, bb * H:(bb + 1) * H],
                                start=first_mm, stop=False,
                                skip_group_check=True,
                            )
                            first_mm = False
                for bb in (BL - 2, BL - 1):
                    for i in tt:
                        stop = (bb == BL - 1 and i == tt[-1])
                        nc.tensor.matmul(
                            ps8[:], e3x[:, bb, :],
                            tail_ht[i][:, tch - 1, bb * H:(bb + 1) * H],
                            start=first_mm, stop=stop,
                            skip_group_check=True,
                        )
                        first_mm = False

            # land the DVE accumulator's per-batch column sums on ps8
            if not pairs and have_dve:
                for bb in range(BL):
                    nc.tensor.matmul(
                        ps8[:], e3m[:, bb, :],
                        acc[:, bb * H:(bb + 1) * H],
                        start=first_mm, stop=(bb == BL - 1),
                        skip_group_check=True,
                    )
                    first_mm = False

            # ---- denom, reciprocal, final scale, store ----
            denq = small.tile([BL, H], f32)
            den = small.tile([BL, 1], f32)
            nc.vector.tensor_mul(out=denq[:], in0=sw_sb[:], in1=ps8[:])
            nc.vector.reduce_sum(out=den[:], in_=denq[:], axis=X)
            inv = small.tile([BL, 1], f32)
            nc.vector.reciprocal(out=inv[:], in_=den[:])
            c_fin = small.tile([BL, H], f32)
            nc.vector.tensor_scalar_mul(out=c_fin[:], in0=ps_o[:], scalar1=inv[:])
            # SP's DGE has the lowest issue+start latency (565+650 ns)
            nc.sync.dma_start(out=out_d[:], in_=c_fin[:])

    _install_birpatch(nc, dedup_ldw=bool(dedup_ldw))
    return nc


def _dma_chunks(nc, eng, ht, h_view, i, tch):
    """Chunked tile DMA (512KB chunks; the final chunk split in quarters,
    the last quarter further into batch-aligned eighths — so after the
    very last 64KB lands only one matmul per tile remains)."""
    Fq = F // 4
    Fe = F // 8
    for c in range(tch):
        if c == tch - 1:
            for q in range(3):
                eng.dma_start(out=ht[:, c, q * Fq:(q + 1) * Fq],
                              in_=h_view[i][:, c, q * Fq:(q + 1) * Fq])
            for e in (6, 7):
                eng.dma_start(out=ht[:, c, e * Fe:(e + 1) * Fe],
                              in_=h_view[i][:, c, e * Fe:(e + 1) * Fe])
        else:
            eng.dma_start(out=ht[:, c, :], in_=h_view[i][:, c, :])


def _get_nc(**kw):
    key = tuple(sorted(kw.items()))
    if key not in _CACHE:
        _CACHE[key] = _build(**kw)
    return _CACHE[key]


def _np_hdt(hprec):
    if hprec == "f16":
        return np.float16
    if hprec == "bf16":
        import ml_dtypes
        return ml_dtypes.bfloat16
    return np.float32


def _diffuse_fp8(h32):
    """Error-diffused fp8e4m3 quantization along t, per (b, h) column:
    q_t = fp8(x_t + e_{t-1}), e_t = the residual. Sum(q) = Sum(x) - e_T,
    so the device's T-sum sees only the final sub-ulp carry per column
    instead of sqrt(T)-accumulated rounding noise."""
    import ml_dtypes
    f8 = ml_dtypes.float8_e4m3   # matches mybir.dt.np(float8e4)
    q = np.empty(h32.shape, dtype=f8)
    e = np.zeros(h32.shape[1:], np.float32)
    for t in range(h32.shape[0]):
        x = h32[t] + e
        qt = x.astype(f8)
        q[t] = qt
        e = x - qt.astype(np.float32)
    return q


def _shard_inputs(s_before, h_sliced, h, W, b, t_total=T, hprec="f8f16"):
    in_maps = []
    if hprec == "f8f16":
        t8 = t_total // 2
        q8 = _diffuse_fp8(h[:t8].astype(np.float32))
        h16 = h[t8:t_total].astype(np.float16)
        for i in range(NCORES):
            sl = slice(i * BL, (i + 1) * BL)
            in_maps.append({
                "h8": np.ascontiguousarray(q8[:, sl, :]).reshape(t8, F),
                "h": np.ascontiguousarray(h16[:, sl, :]).reshape(
                    t_total - t8, F),
                "hs": np.ascontiguousarray(h_sliced[:, sl, :]).reshape(N, F),
                "s": np.ascontiguousarray(s_before[0, sl, :]),
                "w": np.ascontiguousarray(W),
                "bias": np.ascontiguousarray(b).reshape(1, H),
            })
        return in_maps
    ndt = _np_hdt(hprec)
    for i in range(NCORES):
        sl = slice(i * BL, (i + 1) * BL)
        in_maps.append({
            "h": np.ascontiguousarray(
                h[:t_total, sl, :].astype(ndt)).reshape(t_total, F),
            "hs": np.ascontiguousarray(h_sliced[:, sl, :]).reshape(N, F),
            "s": np.ascontiguousarray(s_before[0, sl, :]),
            "w": np.ascontiguousarray(W),
            "bias": np.ascontiguousarray(b).reshape(1, H),
        })
    return in_maps


def _run(s_before, h_sliced, h, W, b, trace=False, **build_kw):
    from concourse.bass_utils import run_bass_kernel_spmd

    nc = _get_nc(**build_kw)
    in_maps = _shard_inputs(s_before, h_sliced, h, W, b,
                            t_total=build_kw.get("t_total", T),
                            hprec=build_kw.get("hprec", "f8f16"))
    bkr = run_bass_kernel_spmd(nc, in_maps, list(range(NCORES)), trace=trace)
    out = np.concatenate([bkr.results[i]["out"] for i in range(NCORES)], axis=0)
    return out, bkr


def kernel(s_before, h_sliced, h, W, b):
    out, _ = _run(
        np.asarray(s_before), np.asarray(h_sliced), np.asarray(h),
        np.asarray(W), np.asarray(b),
    )
    return out
